# revision 9
# baseline (speedup 1.0000x reference)
"""GP prediction kernel for Trainium2 (8 NeuronCores, data-parallel over batch).

Computes z_pred[b, p, d] = sum_c k_mult[p, c] * z_enc[b, c, d] where k_mult
is the [64, 448] GP weight matrix k_pred.T @ inv(cov + sigma*I). k_mult
depends only on compile-time constants, so it is precomputed on host; the
device work is a batched [64,448] @ [448,1024] matmul, sharded 8 batches
per core.

Precision: the correctness gate is rel_err < 2e-2; fp16 operands and fp16
output give ~3.5e-4, so z/k/out all move over HBM as fp16 (half the bytes
of the fp32 baseline).

DMA: the contraction is tiled as 448 = 4 x 112 so one whole batch
([448, 1024] fp16 = 896 KB) lands in SBUF as a single chunked DMA
[112 parts, 4 chunks x 2 KB]. 8 batch loads split across the two HWDGE
queues (sync=SP 4, scalar=ACT 4) exactly fill the per-engine 4-deep
DMA-completion-semaphore window, so the queues stream without issue
stalls. Output stores ride the same queues behind the loads.

PE: batch pairs are column-tiled — batch 2bp in PE columns 0-63, batch
2bp+1 in columns 64-127, accumulating into one [128, 512] PSUM bank —
so the paired matmuls run concurrently and PE keeps up even at the cold
(1.2 GHz) HAM clock.
"""
import numpy as np
from contextlib import ExitStack

import concourse.bacc as bacc
import concourse.tile as tile
from concourse import mybir
from concourse.bass_utils import run_bass_kernel_spmd

# Problem constants (hardcoded per harness contract).
B, T, D = 64, 512, 1024
P = 64                 # N_PREDICTORS
C = T - P              # 448 context timesteps
L, SIGMA, TIMESCALE = 0.01, 0.01, 0.3
N_CORES = 8
BPC = B // N_CORES     # batches per core

KT = 112               # K-tile size: 448 = 4 * 112
NJ = 4                 # K-tiles per batch


def _k_mult_T() -> np.ndarray:
    """[C, P] transpose of the GP weight matrix.

    Replicates the reference's fp32 jax ops on CPU so the constant matches
    the reference's k_mult near-bitwise; falls back to a float64 numpy solve.
    """
    try:
        import jax
        import jax.numpy as jnp

        cpu = jax.devices("cpu")[0]
        with jax.default_device(cpu):
            t = jnp.linspace(0.0, 1.0, T)
            t_in = t[:C] * TIMESCALE
            t_pred = t[C:] * TIMESCALE

            def rbf(x, y):
                d = x[:, None] - y[None, :]
                return jnp.exp(-0.5 * (d * d) / L)

            cov = rbf(t_in, t_in)
            k_pred = rbf(t_in, t_pred)
            eye = jnp.eye(C, dtype=cov.dtype)
            k_mult = k_pred.T @ jnp.linalg.inv(cov + eye * SIGMA)   # [P, C]
            km_T = np.asarray(k_mult).T                             # [C, P]
    except Exception:
        t = np.linspace(0.0, 1.0, T)
        t_in = t[:C] * TIMESCALE
        t_pred = t[C:] * TIMESCALE

        def rbf_np(x, y):
            d = x[:, None] - y[None, :]
            return np.exp(-0.5 * d * d / L)

        cov = rbf_np(t_in, t_in) + np.eye(C) * SIGMA
        km_T = np.linalg.solve(cov, rbf_np(t_in, t_pred))
    return np.ascontiguousarray(km_T.astype(np.float32))


def _km_packed() -> np.ndarray:
    """[KT, NJ*P] fp16: column block j holds K-tile j of k_mult.T."""
    km_T = _k_mult_T().astype(np.float16)      # [C, P]
    out = np.empty((KT, NJ * P), np.float16)
    for j in range(NJ):
        out[:, j * P : (j + 1) * P] = km_T[j * KT : (j + 1) * KT]
    return np.ascontiguousarray(out)


KM_PACKED = _km_packed()

_NC = None


def _build():
    nc = bacc.Bacc()
    z = nc.dram_tensor("z", [BPC * C, D], mybir.dt.float16, kind="ExternalInput")
    km = nc.dram_tensor("km", [KT, NJ * P], mybir.dt.float16, kind="ExternalInput")
    out = nc.dram_tensor("out", [BPC * P, D], mybir.dt.float16, kind="ExternalOutput")

    with tile.TileContext(nc) as tc, ExitStack() as ctx:
        kpool = ctx.enter_context(tc.tile_pool(name="km", bufs=1))
        zpool = ctx.enter_context(tc.tile_pool(name="z", bufs=BPC))
        opool = ctx.enter_context(tc.tile_pool(name="o", bufs=4))
        ppool = ctx.enter_context(tc.tile_pool(name="ps", bufs=4, space="PSUM"))

        km_sb = kpool.tile([KT, NJ * P], mybir.dt.float16)
        nc.scalar.dma_start(km_sb[:, :], km[:, :])

        def km_j(j):
            return km_sb[:, j * P : (j + 1) * P]

        # Phase 1: one chunked 896 KB DMA per batch, 4 per HWDGE queue.
        zt = {}
        for b in range(BPC):
            zt[b] = zpool.tile([KT, NJ * D], mybir.dt.float16,
                               name=f"zt{b}", tag="zt")
            src = z[b * C : (b + 1) * C, :].rearrange(
                "(c p) d -> p c d", c=NJ, p=KT
            )
            dst = zt[b][:, :].rearrange("p (c d) -> p c d", c=NJ, d=D)
            eng = nc.sync if b % 2 == 0 else nc.scalar
            eng.dma_start(dst, src)

        # Phase 2: column-tiled batch pairs; j outermost so one weight tile
        # serves 4 consecutive matmuls.
        for bp in range(BPC // 2):
            out_sb = opool.tile([128, D], mybir.dt.float16, name=f"osb{bp}",
                                tag="osb")
            pss = [
                ppool.tile([128, 512], mybir.dt.float32, name=f"ps{bp}_{n}",
                           tag="ps")
                for n in range(2)
            ]
            for j in range(NJ):
                for n in range(2):
                    for half in range(2):
                        nc.tensor.matmul(
                            pss[n][half * P : (half + 1) * P, :],
                            km_j(j),
                            zt[2 * bp + half][:, j * D + n * 512
                                              : j * D + n * 512 + 512],
                            start=(j == 0), stop=(j == NJ - 1),
                            tile_position=(0, half * P),
                        )
            for n in range(2):
                nc.vector.tensor_copy(
                    out_sb[:, n * 512 : (n + 1) * 512], pss[n][:, :]
                )
            oeng = nc.sync if bp % 2 == 0 else nc.scalar
            oeng.dma_start(out[bp * 128 : (bp + 1) * 128, :], out_sb[:])

    nc.finalize()
    return nc


def kernel(z_enc: np.ndarray, _trace: bool = False):
    global _NC
    z_enc = np.asarray(z_enc, dtype=np.float32)
    if _NC is None:
        _NC = _build()

    z16 = z_enc[:, :C, :].astype(np.float16)
    in_maps = []
    for i in range(N_CORES):
        shard = z16[i * BPC : (i + 1) * BPC].reshape(BPC * C, D)
        in_maps.append({"z": shard, "km": KM_PACKED})

    res = run_bass_kernel_spmd(_NC, in_maps, core_ids=list(range(N_CORES)),
                               trace=_trace)
    out = np.concatenate(
        [r["out"].astype(np.float32).reshape(BPC, P, D) for r in res.results],
        axis=0,
    )
    if _trace:
        return out, res
    return out
